# revision 2
# baseline (speedup 1.0000x reference)
"""Trainium2 Bass kernel for a 2-layer GAT (DGL-style GATConv, eval mode).

Strategy (8 NeuronCores, SPMD):
  - Edges are sorted by dst on the host and sharded by contiguous dst range,
    so each core owns a disjoint set of destination nodes and needs no
    cross-core reduction for the aggregation itself.
  - Per-node tables T = [feat | el] are built on-device from the core's node
    shard (x @ W, then per-head dots with attn vectors) and AllGathered so
    every core can gather rows for its edges' arbitrary src ids.
  - Edge phase runs over 128-node windows; per 128-edge tile it
    indirect-DMA-gathers T[src] and er[dst], computes ex = exp(leaky_relu
    (el+er)) (segment-max is skipped: softmax is shift-invariant and the
    logits are O(1) here), builds a node-selector 0/1 matrix with one
    is_equal against an iota tile, and does a single bf16 matmul
    sel.T @ [ex*feat | ex] that accumulates per-window node sums in PSUM.
    At window end the sums are normalized by the gathered denominator and
    written out contiguously - no scatter DMAs anywhere.
  - Padding edges point at a padding node whose el is forced to -1e9 by a
    data-driven mask, so exp(leaky_relu(...)) == 0 and they are neutral.
"""

import os
import sys

for _p in ("/opt/trn_rl_repo",):
    if _p not in sys.path:
        sys.path.insert(0, _p)

import numpy as np

import concourse.bacc as bacc
import concourse.bass as bass
import concourse.tile as tile
from concourse import mybir
from concourse.bass_utils import run_bass_kernel_spmd
from concourse.masks import make_identity

N_CORES = 8
P = 128
NEG_SLOPE = 0.2
EPS = 1e-30
MASK_NEG = -1.0e9

FP32 = mybir.dt.float32
BF16 = mybir.dt.bfloat16
I32 = mybir.dt.int32


# ---------------------------------------------------------------- host side


def _schedule(dst: np.ndarray, n_nodes: int):
    """Sort/shard/pack edges. Returns (order, tw, meta_pos, dims) where
    order is the edge permutation, tw[w] the (core-equalized) tile count of
    window w, and meta_pos the flat slot of each sorted edge in its core's
    meta array."""
    sh = int(np.ceil(n_nodes / (N_CORES * P))) * P
    if N_CORES * sh == n_nodes:
        sh += P  # ensure at least one padding node exists
    n_pad = N_CORES * sh
    w_per_core = sh // P

    order = np.argsort(dst, kind="stable")
    dsts = dst[order].astype(np.int64)
    core = dsts // sh
    win = (dsts % sh) // P

    # counts[c, w]
    counts = np.zeros((N_CORES, w_per_core), dtype=np.int64)
    flat = core * w_per_core + win
    binc = np.bincount(flat, minlength=N_CORES * w_per_core)
    counts[:, :] = binc.reshape(N_CORES, w_per_core)

    tw = np.maximum(1, -(-counts // P)).max(axis=0)  # [w_per_core]
    t_start = np.concatenate([[0], np.cumsum(tw)])  # slot offset per window
    t_total = int(t_start[-1])

    # position of each sorted edge inside its (core, win) group
    grp_starts = np.zeros(N_CORES * w_per_core + 1, dtype=np.int64)
    np.cumsum(binc, out=grp_starts[1:])
    rank = np.arange(len(dsts), dtype=np.int64) - grp_starts[flat]
    meta_pos = t_start[win] * P + rank  # flat slot within the core
    return order, core, tw, meta_pos, dict(
        sh=sh, n_pad=n_pad, w_per_core=w_per_core, t_total=t_total,
        t_start=t_start,
    )


def _pack_meta(src, dst, order, core, meta_pos, dims):
    """Build per-core meta arrays [P, 3*t_total] int32.
    slot 3t+0: global src id; 3t+1: core-local dst id (for er gather);
    3t+2: f32 bits of dst id within the 128-node window."""
    sh, t_total = dims["sh"], dims["t_total"]
    t_start, wpc = dims["t_start"], dims["w_per_core"]
    pad_src = dims["n_pad"] - 1

    # defaults for padding slots
    win_of_slot = np.zeros(t_total, dtype=np.int64)
    for w in range(wpc):
        win_of_slot[t_start[w]:t_start[w + 1]] = w

    metas = []
    for c in range(N_CORES):
        m = np.empty((t_total * P, 3), dtype=np.int32)
        m[:, 0] = pad_src
        m[:, 1] = np.repeat(win_of_slot * P, P).astype(np.int32)
        m[:, 2] = np.float32(0.0).view(np.int32)

        sel = core == c
        e = order[sel]
        pos = meta_pos[sel]
        d_loc = (dst[e].astype(np.int64) - c * sh).astype(np.int32)
        m[pos, 0] = src[e].astype(np.int32)
        m[pos, 1] = d_loc
        m[pos, 2] = (d_loc % P).astype(np.float32).view(np.int32)

        # -> [P, 3*t_total]: slot (t, p) holds cols 3t..3t+2 of partition p
        m = m.reshape(t_total, P, 3).transpose(1, 0, 2).reshape(P, 3 * t_total)
        metas.append(np.ascontiguousarray(m))
    return metas


# -------------------------------------------------------------- device side


def _build_program(cfg):
    """Build the SPMD Bacc program. cfg keys:
    in_dim, f1, h1, hid, f2, h2, outd, sh, n_pad, wpc, tw (tuple), t_total.
    """
    in_dim, f1, h1, hid = cfg["in_dim"], cfg["f1"], cfg["h1"], cfg["hid"]
    f2, h2, outd = cfg["f2"], cfg["h2"], cfg["outd"]
    sh, n_pad, wpc = cfg["sh"], cfg["n_pad"], cfg["wpc"]
    tw, t_total = cfg["tw"], cfg["t_total"]
    c1, c2 = f1 + h1, f2 + h2
    kc = in_dim // P
    assert in_dim % P == 0 and f1 == P and h2 == 1

    nc = bacc.Bacc("TRN2", target_bir_lowering=False, debug=False,
                   num_devices=N_CORES)

    # I/O
    x_sh = nc.dram_tensor("x_shard", [sh, in_dim], FP32, kind="ExternalInput")
    w1_d = nc.dram_tensor("w1", [in_dim, f1], FP32, kind="ExternalInput")
    w2_d = nc.dram_tensor("w2", [f1, f2], FP32, kind="ExternalInput")
    al1_d = nc.dram_tensor("al1f", [1, f1], FP32, kind="ExternalInput")
    ar1_d = nc.dram_tensor("ar1f", [1, f1], FP32, kind="ExternalInput")
    b1_d = nc.dram_tensor("b1f", [1, f1], FP32, kind="ExternalInput")
    al2_d = nc.dram_tensor("al2f", [1, f2], FP32, kind="ExternalInput")
    ar2_d = nc.dram_tensor("ar2f", [1, f2], FP32, kind="ExternalInput")
    b2_d = nc.dram_tensor("b2f", [1, f2], FP32, kind="ExternalInput")
    mask_d = nc.dram_tensor("elmask", [sh, 1], FP32, kind="ExternalInput")
    meta_d = nc.dram_tensor("meta", [P, 3 * t_total], I32,
                            kind="ExternalInput")
    out_d = nc.dram_tensor("out", [sh, outd], FP32, kind="ExternalOutput")

    # internal DRAM
    t1_sh = nc.dram_tensor("t1_shard", [sh, c1], FP32)
    er1_sh = nc.dram_tensor("er1_shard", [sh, h1], FP32)
    t2_sh = nc.dram_tensor("t2_shard", [sh, c2], FP32)
    er2_sh = nc.dram_tensor("er2_shard", [sh, h2], FP32)
    h1_sh = nc.dram_tensor("h1_shard", [sh, f1], FP32)
    t1_full = nc.dram_tensor("t1_full", [n_pad, c1], FP32,
                             addr_space="Shared")
    t2_full = nc.dram_tensor("t2_full", [n_pad, c2], FP32,
                             addr_space="Shared")
    rg = [list(range(N_CORES))]

    with tile.TileContext(nc) as tc:
        with tc.tile_pool(name="const", bufs=1) as cpool, \
             tc.tile_pool(name="work", bufs=3) as wp, \
             tc.tile_pool(name="io", bufs=3) as iop, \
             tc.tile_pool(name="psum", bufs=2, space="PSUM") as pp:

            # ---- constants
            ident = cpool.tile([P, P], FP32)
            make_identity(nc, ident[:])
            iota_i = cpool.tile([P, P], I32)
            nc.gpsimd.iota(iota_i[:], pattern=[[1, P]], channel_multiplier=0)
            iota_f = cpool.tile([P, P], FP32)
            nc.vector.tensor_copy(out=iota_f[:], in_=iota_i[:])

            w1_sb = []
            for k in range(kc):
                t = cpool.tile([P, f1], FP32, tag=f"w1_{k}")
                nc.sync.dma_start(out=t[:], in_=w1_d[k * P:(k + 1) * P, :])
                w1_sb.append(t)
            w2_sb = cpool.tile([P, f2], FP32)
            nc.sync.dma_start(out=w2_sb[:], in_=w2_d[:, :])

            def bcast_const(dram, width, tag):
                t = cpool.tile([P, width], FP32, tag=tag)
                nc.sync.dma_start(out=t[:],
                                  in_=dram[0:1, :].to_broadcast([P, width]))
                return t

            al1_bc = bcast_const(al1_d, f1, "al1")
            ar1_bc = bcast_const(ar1_d, f1, "ar1")
            b1_bc = bcast_const(b1_d, f1, "b1")
            al2_bc = bcast_const(al2_d, f2, "al2")
            ar2_bc = bcast_const(ar2_d, f2, "ar2")
            b2_bc = bcast_const(b2_d, f2, "b2")

            # ---- node phase 1: T1 = [x@W1 | el1(+mask)], er1
            for nt in range(wpc):
                r0 = nt * P
                x_sb = iop.tile([P, in_dim], FP32, tag="x_in")
                nc.sync.dma_start(out=x_sb[:], in_=x_sh[r0:r0 + P, :])
                msk = iop.tile([P, 1], FP32, tag="msk")
                nc.sync.dma_start(out=msk[:], in_=mask_d[r0:r0 + P, :])

                feat_ps = pp.tile([P, f1], FP32, tag="feat_ps")
                for k in range(kc):
                    xt_ps = pp.tile([P, P], FP32, tag="xt_ps")
                    nc.tensor.transpose(out=xt_ps[:],
                                        in_=x_sb[:, k * P:(k + 1) * P],
                                        identity=ident[:])
                    xt_sb = wp.tile([P, P], FP32, tag="xt_sb")
                    nc.vector.tensor_copy(out=xt_sb[:], in_=xt_ps[:])
                    nc.tensor.matmul(feat_ps[:], xt_sb[:], w1_sb[k][:],
                                     start=(k == 0), stop=(k == kc - 1))

                t1_st = wp.tile([P, c1], FP32, tag="t1_st")
                nc.vector.tensor_copy(out=t1_st[:, :f1], in_=feat_ps[:])
                tmp = wp.tile([P, f1], FP32, tag="np1_tmp")
                el = wp.tile([P, h1], FP32, tag="np1_el")
                nc.vector.tensor_tensor(out=tmp[:], in0=feat_ps[:],
                                        in1=al1_bc[:],
                                        op=mybir.AluOpType.mult)
                nc.vector.tensor_reduce(
                    out=el[:], in_=tmp[:].rearrange("p (h d) -> p h d", h=h1),
                    axis=mybir.AxisListType.X, op=mybir.AluOpType.add)
                nc.vector.tensor_tensor(out=t1_st[:, f1:c1], in0=el[:],
                                        in1=msk[:].to_broadcast([P, h1]),
                                        op=mybir.AluOpType.add)
                er = wp.tile([P, h1], FP32, tag="np1_er")
                nc.vector.tensor_tensor(out=tmp[:], in0=feat_ps[:],
                                        in1=ar1_bc[:],
                                        op=mybir.AluOpType.mult)
                nc.vector.tensor_reduce(
                    out=er[:], in_=tmp[:].rearrange("p (h d) -> p h d", h=h1),
                    axis=mybir.AxisListType.X, op=mybir.AluOpType.add)
                nc.sync.dma_start(out=t1_sh[r0:r0 + P, :], in_=t1_st[:])
                nc.sync.dma_start(out=er1_sh[r0:r0 + P, :], in_=er[:])

            tc.strict_bb_all_engine_barrier()
            nc.gpsimd.collective_compute(
                "AllGather", mybir.AluOpType.bypass,
                ins=[t1_sh.ap().opt()], outs=[t1_full.ap().opt()],
                replica_groups=rg)
            tc.strict_bb_all_engine_barrier()

            # ---- edge phase 1 -> h1_shard
            def edge_phase(t_tab, er_tab, cc, ff, hh, dd, finish):
                t0 = 0
                for w in range(wpc):
                    ntw = tw[w]
                    meta_sb = iop.tile([P, 3 * ntw], I32, tag="meta_sb")
                    nc.sync.dma_start(
                        out=meta_sb[:],
                        in_=meta_d[:, 3 * t0:3 * (t0 + ntw)])
                    ps = pp.tile([P, cc], FP32, tag="agg_ps")
                    for j in range(ntw):
                        src_idx = meta_sb[:, 3 * j:3 * j + 1]
                        dloc_idx = meta_sb[:, 3 * j + 1:3 * j + 2]
                        dwin_f = meta_sb[:, 3 * j + 2:3 * j + 3].bitcast(FP32)

                        g = wp.tile([P, cc], FP32, tag="gat")
                        nc.gpsimd.indirect_dma_start(
                            out=g[:], out_offset=None, in_=t_tab[:, :],
                            in_offset=bass.IndirectOffsetOnAxis(
                                ap=src_idx, axis=0))
                        ere = wp.tile([P, hh], FP32, tag="ere")
                        nc.gpsimd.indirect_dma_start(
                            out=ere[:], out_offset=None, in_=er_tab[:, :],
                            in_offset=bass.IndirectOffsetOnAxis(
                                ap=dloc_idx, axis=0))

                        e_sb = wp.tile([P, hh], FP32, tag="e_sb")
                        nc.vector.tensor_tensor(out=e_sb[:], in0=g[:, ff:cc],
                                                in1=ere[:],
                                                op=mybir.AluOpType.add)
                        # leaky_relu(x) = max(0.2*x, x)
                        nc.vector.scalar_tensor_tensor(
                            out=e_sb[:], in0=e_sb[:], scalar=NEG_SLOPE,
                            in1=e_sb[:], op0=mybir.AluOpType.mult,
                            op1=mybir.AluOpType.max)
                        ex = wp.tile([P, hh], FP32, tag="ex")
                        nc.scalar.activation(
                            out=ex[:], in_=e_sb[:],
                            func=mybir.ActivationFunctionType.Exp)

                        sel = wp.tile([P, P], BF16, tag="sel")
                        nc.vector.tensor_tensor(
                            out=sel[:], in0=dwin_f.to_broadcast([P, P]),
                            in1=iota_f[:], op=mybir.AluOpType.is_equal)

                        m = wp.tile([P, cc], BF16, tag="m")
                        nc.vector.tensor_tensor(
                            out=m[:, :ff].rearrange("p (h d) -> p h d", h=hh),
                            in0=g[:, :ff].rearrange("p (h d) -> p h d", h=hh),
                            in1=ex[:].to_broadcast([P, hh, dd]),
                            op=mybir.AluOpType.mult)
                        nc.vector.tensor_copy(out=m[:, ff:cc], in_=ex[:])

                        nc.tensor.matmul(ps[:], sel[:], m[:],
                                         start=(j == 0), stop=(j == ntw - 1))
                    finish(w, ps)
                    t0 += ntw

            def finish1(w, ps):
                den = wp.tile([P, h1], FP32, tag="den")
                nc.vector.tensor_scalar_add(den[:], ps[:, f1:c1], EPS)
                rec = wp.tile([P, h1], FP32, tag="rec")
                nc.vector.reciprocal(rec[:], den[:])
                o = wp.tile([P, f1], FP32, tag="o1")
                nc.vector.tensor_tensor(
                    out=o[:].rearrange("p (h d) -> p h d", h=h1),
                    in0=ps[:, :f1].rearrange("p (h d) -> p h d", h=h1),
                    in1=rec[:].to_broadcast([P, h1, hid]),
                    op=mybir.AluOpType.mult)
                nc.vector.tensor_tensor(out=o[:], in0=o[:], in1=b1_bc[:],
                                        op=mybir.AluOpType.add)
                # ELU(x) = exp(min(x,0)) - 1 + max(x,0)
                tneg = wp.tile([P, f1], FP32, tag="tneg")
                nc.vector.tensor_scalar_min(tneg[:], o[:], 0.0)
                texp = wp.tile([P, f1], FP32, tag="texp")
                nc.scalar.activation(out=texp[:], in_=tneg[:],
                                     func=mybir.ActivationFunctionType.Exp)
                nc.vector.tensor_scalar_max(o[:], o[:], 0.0)
                h1v = wp.tile([P, f1], FP32, tag="h1v")
                nc.vector.scalar_tensor_tensor(
                    out=h1v[:], in0=texp[:], scalar=-1.0, in1=o[:],
                    op0=mybir.AluOpType.add, op1=mybir.AluOpType.add)
                nc.sync.dma_start(out=h1_sh[w * P:(w + 1) * P, :], in_=h1v[:])

            edge_phase(t1_full, er1_sh, c1, f1, h1, hid, finish1)

            tc.strict_bb_all_engine_barrier()

            # ---- node phase 2: T2 = [h1@W2 | el2(+mask)], er2
            for nt in range(wpc):
                r0 = nt * P
                hsb = iop.tile([P, f1], FP32, tag="h_in")
                nc.sync.dma_start(out=hsb[:], in_=h1_sh[r0:r0 + P, :])
                msk = iop.tile([P, 1], FP32, tag="msk")
                nc.sync.dma_start(out=msk[:], in_=mask_d[r0:r0 + P, :])

                ht_ps = pp.tile([P, P], FP32, tag="xt_ps")
                nc.tensor.transpose(out=ht_ps[:], in_=hsb[:],
                                    identity=ident[:])
                ht_sb = wp.tile([P, P], FP32, tag="xt_sb")
                nc.vector.tensor_copy(out=ht_sb[:], in_=ht_ps[:])
                f2_ps = pp.tile([P, f2], FP32, tag="f2_ps")
                nc.tensor.matmul(f2_ps[:], ht_sb[:], w2_sb[:],
                                 start=True, stop=True)

                t2_st = wp.tile([P, c2], FP32, tag="t2_st")
                nc.vector.tensor_copy(out=t2_st[:, :f2], in_=f2_ps[:])
                tmp2 = wp.tile([P, f2], FP32, tag="np2_tmp")
                el2 = wp.tile([P, h2], FP32, tag="np2_el")
                nc.vector.tensor_tensor(out=tmp2[:], in0=f2_ps[:],
                                        in1=al2_bc[:],
                                        op=mybir.AluOpType.mult)
                nc.vector.tensor_reduce(out=el2[:], in_=tmp2[:],
                                        axis=mybir.AxisListType.X,
                                        op=mybir.AluOpType.add)
                nc.vector.tensor_tensor(out=t2_st[:, f2:c2], in0=el2[:],
                                        in1=msk[:].to_broadcast([P, h2]),
                                        op=mybir.AluOpType.add)
                er2 = wp.tile([P, h2], FP32, tag="np2_er")
                nc.vector.tensor_tensor(out=tmp2[:], in0=f2_ps[:],
                                        in1=ar2_bc[:],
                                        op=mybir.AluOpType.mult)
                nc.vector.tensor_reduce(out=er2[:], in_=tmp2[:],
                                        axis=mybir.AxisListType.X,
                                        op=mybir.AluOpType.add)
                nc.sync.dma_start(out=t2_sh[r0:r0 + P, :], in_=t2_st[:])
                nc.sync.dma_start(out=er2_sh[r0:r0 + P, :], in_=er2[:])

            tc.strict_bb_all_engine_barrier()
            nc.gpsimd.collective_compute(
                "AllGather", mybir.AluOpType.bypass,
                ins=[t2_sh.ap().opt()], outs=[t2_full.ap().opt()],
                replica_groups=rg)
            tc.strict_bb_all_engine_barrier()

            # ---- edge phase 2 -> out (h2 == 1: mean over heads is identity)
            def finish2(w, ps):
                den = wp.tile([P, h2], FP32, tag="den2")
                nc.vector.tensor_scalar_add(den[:], ps[:, f2:c2], EPS)
                rec = wp.tile([P, h2], FP32, tag="rec2")
                nc.vector.reciprocal(rec[:], den[:])
                o = wp.tile([P, f2], FP32, tag="o2")
                nc.vector.tensor_tensor(out=o[:], in0=ps[:, :f2],
                                        in1=rec[:].to_broadcast([P, f2]),
                                        op=mybir.AluOpType.mult)
                nc.vector.tensor_tensor(out=o[:], in0=o[:], in1=b2_bc[:],
                                        op=mybir.AluOpType.add)
                nc.sync.dma_start(out=out_d[w * P:(w + 1) * P, :], in_=o[:])

            edge_phase(t2_full, er2_sh, c2, f2, h2, outd, finish2)

    nc.compile()
    return nc


# ------------------------------------------------------------------- driver

_CACHE = {}


def _prepare(x, W1, al1, ar1, b1, W2, al2, ar2, b2, src, dst):
    n, in_dim = x.shape
    h1, hid = al1.shape
    f1 = W1.shape[1]
    h2, outd = al2.shape
    f2 = W2.shape[1]
    assert f1 == h1 * hid and f2 == h2 * outd

    order, core, tw, meta_pos, dims = _schedule(dst, n)
    metas = _pack_meta(src, dst, order, core, meta_pos, dims)
    sh, n_pad, wpc = dims["sh"], dims["n_pad"], dims["w_per_core"]

    cfg = dict(in_dim=in_dim, f1=f1, h1=h1, hid=hid, f2=f2, h2=h2,
               outd=outd, sh=sh, n_pad=n_pad, wpc=wpc,
               tw=tuple(int(v) for v in tw), t_total=dims["t_total"])

    x_pad = np.zeros((n_pad, in_dim), dtype=np.float32)
    x_pad[:n] = np.asarray(x, dtype=np.float32)
    gids = np.arange(n_pad)
    mask_full = np.where(gids < n, 0.0, MASK_NEG).astype(np.float32)[:, None]

    common = {
        "w1": np.asarray(W1, np.float32),
        "w2": np.asarray(W2, np.float32),
        "al1f": np.asarray(al1, np.float32).reshape(1, f1),
        "ar1f": np.asarray(ar1, np.float32).reshape(1, f1),
        "b1f": np.asarray(b1, np.float32).reshape(1, f1),
        "al2f": np.asarray(al2, np.float32).reshape(1, f2),
        "ar2f": np.asarray(ar2, np.float32).reshape(1, f2),
        "b2f": np.asarray(b2, np.float32).reshape(1, f2),
    }
    in_maps = []
    for c in range(N_CORES):
        m = dict(common)
        m["x_shard"] = np.ascontiguousarray(x_pad[c * sh:(c + 1) * sh])
        m["elmask"] = np.ascontiguousarray(mask_full[c * sh:(c + 1) * sh])
        m["meta"] = metas[c]
        in_maps.append(m)
    return cfg, in_maps, n, sh, outd


def build_and_run(inputs, trace=False, trace_kwargs=None):
    cfg, in_maps, n, sh, outd = _prepare(**inputs)
    key = tuple(sorted(cfg.items(), key=lambda kv: kv[0]))
    key = repr(key)
    if key not in _CACHE:
        _CACHE[key] = _build_program(cfg)
    nc = _CACHE[key]
    res = run_bass_kernel_spmd(
        nc, in_maps, core_ids=list(range(N_CORES)), trace=trace,
        **(trace_kwargs or {}))
    out = np.concatenate([res.results[c]["out"] for c in range(N_CORES)],
                         axis=0)[:n]
    return np.ascontiguousarray(out.astype(np.float32)), res


def kernel(**inputs) -> np.ndarray:
    out, _ = build_and_run(inputs, trace=False)
    return out
